# revision 1
# baseline (speedup 1.0000x reference)
"""AttentionSink masked-add kernel for 8 TRN2 NeuronCores.

out[b,h,i,j] = w[b,h,i,j] + mask[i,j], mask 0 where allowed else -1e30.
Allowed: j < 4 (sink) or i-25 <= j <= i (local band).

Since |w| << ulp(-1e30) in fp32, masked outputs are exactly -1e30.

The kernel works in a TRANSPOSED, MATRIX-INTERLEAVED per-core layout
T[j, i, m] = out[m, i, j] (the host permutes each shard on the way in
and returns a permuted view on the way out). Two effects:
  - transpose: the mask keeps its diagonal-band structure (allowed:
    T row j, cols j..j+25) but the 4 sink columns become 4 contiguous
    ROWS T[0:4] (one DRAM->DRAM copy, 64 KB descriptors) and the band
    clip collapses to a 26x26 corner;
  - m-interleave: every row element holds all 8 matrices contiguously,
    so the 26-element band becomes 832-byte descriptors (past the
    sub-512 B descriptor penalty) and the wrap-around constant chunks
    become 64.7 KB contiguous runs.
Wrap chunks: for row j in [4, 2021], the masked span
[row j, elems (j+26)*M..] ++ [row j+1, elems 0..(j+1)*M) is one
contiguous (S-25)*M-element run at flat offset j*M*(S+1) + 26*M,
stride M*(S+1) between rows; sourced from a broadcast SBUF constant
row as 8 descriptors of 8092 B per chunk. Zero overlap with the band:
every output byte is written exactly once (the cover is exact).

The corner const rect is split so a no-dep piece is in flight while
the band copy's sem-propagation chain (~2.3 us) resolves, and the
sink-row + corner loads fill the startup while the constant row is
memset: the TimelineSim DMA schedule is gapless end to end.

Per-core HBM traffic: 134.2 MB written (the output size exactly) +
~2.1 MB read. The 64 (S,S) matrices are split 8 per core.
"""

import sys

import numpy as np

try:
    import concourse.bass as bass
except ImportError:  # fresh environment: add the repo staging paths
    for p in ("/opt/trn_rl_repo", "/root/.axon_site/_ro/trn_rl_repo"):
        if p not in sys.path:
            sys.path.append(p)
    import concourse.bass as bass

import concourse.tile as tile
from concourse import bacc, mybir
from concourse.bass_utils import run_bass_kernel_spmd

B, H, S = 4, 16, 2048
SINK = 4
LEFT = 25
NEG = -1e30
P = 128                    # SBUF partitions / max rows per chunk block
N_CORES = 8
M = (B * H) // N_CORES     # matrices per core
CLEN = S - LEFT            # 2023: wrap-chunk length per matrix
J0 = SINK                  # 4: first wrap-chunk row
JE = S - LEFT - 2          # 2021: last wrap-chunk row (j+26 <= 2047)
CC = LEFT + 1              # 26: clipped corner size (rows/cols 2022..2047)
RS = M * (S + 1)           # flat stride between diagonal rows


def _build_program():
    nc = bacc.Bacc(
        "TRN2", target_bir_lowering=False, debug=False, num_devices=N_CORES
    )
    dt = mybir.dt.float32
    # T[j, i, m]: transposed, matrix-interleaved
    x = nc.dram_tensor("x", [S, S, M], dt, kind="ExternalInput").ap()
    out = nc.dram_tensor("out", [S, S, M], dt, kind="ExternalOutput").ap()

    with tile.TileContext(nc) as tc:
        with tc.tile_pool(name="pool", bufs=1) as pool:
            # no-dep transfers first: they fill the DMA device while the
            # constant row is memset.
            # sink rows 0..3: contiguous passthrough, 64 KB descriptors
            nc.sync.dma_start(out[0:SINK], x[0:SINK])
            # band tail piece (rows 1796..2021) as the scalar queue's head
            # filler: DRAM->DRAM, memset-independent; it only bbox-overlaps
            # the late chunk blocks, whose waits resolve ~300 us early.
            bsplit = J0 + 14 * P  # 1796
            tb_off = bsplit * RS
            tb_dims = [[RS, JE + 1 - bsplit], [1, CC * M]]
            nc.scalar.dma_start(
                bass.AP(out.tensor, tb_off, tb_dims),
                bass.AP(x.tensor, tb_off, tb_dims),
            )
            # 26x26(xM) clipped-corner load -> [26, 26*M] SBUF tile
            ct = pool.tile([CC, CC * M], dt, name="ct")
            nc.scalar.dma_start(
                ct[:],
                bass.AP(
                    x.tensor,
                    (S - CC) * S * M + (S - CC) * M,
                    [[S * M, CC], [1, CC * M]],
                ),
            )

            # constant -1e30 source row, memset split across two engines
            c = pool.tile([P, CLEN], dt, name="c")
            nc.vector.memset(c[:, 0:934], NEG)
            nc.gpsimd.memset(c[:, 934:CLEN], NEG)

            def cbc(rows, n, ln, r0=0):
                # c[r0:r0+rows] broadcast: (rows, n, ln) with stride-0 mid
                a = c[r0 : r0 + rows, 0:ln]
                (ps, pn), (ws, wn) = a.ap
                return bass.AP(a.tensor, a.offset, [[ps, pn], [0, n], [1, ln]])

            # corner mask: T row j = 2022+p allows i in [j, j+25];
            # locally (p, k = i - 2022, m): keep k >= p (k <= p+25 always
            # holds for k < 26). One affine select over free dims (k, m).
            nc.gpsimd.affine_select(
                ct[:],
                ct[:],
                [[1, CC], [0, M]],
                mybir.AluOpType.is_ge,
                NEG,
                base=0,
                channel_multiplier=-1,
            )

            # early no-dep rect piece: rows 2023..2027, elems 0..2022*M
            nc.scalar.dma_start(
                bass.AP(
                    out.tensor,
                    (S - CC + 1) * S * M,
                    [[S * M, 5], [JE + 1, M], [1, JE + 1]],
                ),
                cbc(5, M, JE + 1),
            )
            # row 4, cols 0..3 (x M): tiny const piece no chunk covers
            nc.scalar.dma_start(
                bass.AP(out.tensor, J0 * S * M, [[1, 1], [1, SINK * M]]),
                bass.AP(c[:].tensor, c[:].offset, [[1, 1], [1, SINK * M]]),
            )

            # wrap-around const chunks, <=128-row blocks, alternating rings
            for bi, j0 in enumerate(range(J0, JE + 1, P)):
                rows = min(P, JE + 1 - j0)
                off = j0 * RS + CC * M
                dims = [[RS, rows], [CLEN, M], [1, CLEN]]
                ceng = nc.sync if bi % 2 == 0 else nc.scalar
                ceng.dma_start(bass.AP(out.tensor, off, dims), cbc(rows, M, CLEN))

            # late no-dep rect piece (rows 2028..2047): in flight while the
            # band copy's wait on the chunks resolves
            nc.scalar.dma_start(
                bass.AP(
                    out.tensor,
                    (S - CC + 6) * S * M,
                    [[S * M, CC - 6], [JE + 1, M], [1, JE + 1]],
                ),
                cbc(CC - 6, M, JE + 1, r0=5),
            )

            # band rows 4..1795: one DRAM->DRAM diagonal copy, 832 B descs
            # (rows 1796..2021 were the startup filler above)
            boff = J0 * RS
            bdims = [[RS, bsplit - J0], [1, CC * M]]
            nc.sync.dma_start(
                bass.AP(out.tensor, boff, bdims),
                bass.AP(x.tensor, boff, bdims),
            )

            # clipped-corner store (rows/cols 2022..2047)
            nc.scalar.dma_start(
                bass.AP(
                    out.tensor,
                    (S - CC) * S * M + (S - CC) * M,
                    [[S * M, CC], [1, CC * M]],
                ),
                ct[:],
            )

    nc.compile()
    return nc


_CACHE = {}


def _get_nc():
    if "nc" not in _CACHE:
        _CACHE["nc"] = _build_program()
    return _CACHE["nc"]


def _in_maps(w):
    # device layout: T[j, i, m] = w[m, i, j] per 8-matrix shard
    flat = np.asarray(w, dtype=np.float32).reshape(B * H, S, S)
    return [
        {
            "x": np.ascontiguousarray(
                flat[i * M : (i + 1) * M].transpose(2, 1, 0)
            )
        }
        for i in range(N_CORES)
    ]


def _gather(chunks):
    """Stack per-core (S,S,M) results along axis 0. Zero-copy when they are
    consecutive contiguous slices of one base buffer (bass2jax returns views
    of a single concatenated array); otherwise fall back to a copy."""
    try:
        c0 = chunks[0]
        step = c0.nbytes
        ptr0 = c0.__array_interface__["data"][0]
        base = c0.base
        if base is not None and all(
            c.base is base
            and c.flags["C_CONTIGUOUS"]
            and c.__array_interface__["data"][0] == ptr0 + i * step
            for i, c in enumerate(chunks)
        ):
            # one shared owner + adjacent layout: a strided view over c0
            # (whose .base keeps the owner alive) covers all of them
            return np.lib.stride_tricks.as_strided(
                c0,
                shape=(len(chunks),) + c0.shape,
                strides=(step,) + c0.strides,
            )
    except Exception:
        pass
    return np.concatenate([c[None] for c in chunks], axis=0)


def kernel(attention_weights, seq_len=None):
    w = np.asarray(attention_weights, dtype=np.float32)
    assert w.shape == (B, H, S, S)
    nc = _get_nc()
    res = run_bass_kernel_spmd(nc, _in_maps(w), core_ids=list(range(N_CORES)))
    out_t = _gather([res.results[i]["out"] for i in range(N_CORES)])
    # out_t: (N_CORES, S, S, M) with out_t[c, j, i, m] = out[c*M+m, i, j];
    # permute back to (B, H, S, S) as a view
    return (
        out_t.transpose(0, 3, 2, 1)
        .reshape(B, H, S, S)
    )

